# revision 10
# baseline (speedup 1.0000x reference)
"""MoE MLP (top-2 of 8 experts) Trainium2 kernel.

Strategy: expert-parallel across the 8 NeuronCores (host does the exact fp32
top-2 gating and per-expert token gather, as before), but both big matmuls run
as fp8e4m3 DoubleRow with a hi/lo split:

    v = v_hi + v_lo,  v_hi = e4m3(v),  v_lo = e4m3(v - v_hi)

Per 256 contraction rows, three DoubleRow passes accumulate into PSUM:
    speed pass : pairs (w_hi[2u], w_hi[2u+1]) x (x_hi[2u], x_hi[2u+1])
    cross pass : pairs (w_hi[k],  w_lo[k])   x (x_lo[k],  x_hi[k])   (x2)
which computes sum(x_hi*w_hi + x_lo*w_hi + x_hi*w_lo) - exact up to the
dropped lo*lo term (~1e-3 relative). DoubleRow fp8 runs the PE at 2x fp16
rate per pass, so 3 passes per 256 rows = 0.75x the fp16 matmul time, with
near-fp16 accuracy (measured rel err ~2e-3 end to end).

SBUF plane layout (no strided pair slices needed; hi/lo halves are grouped
so every cross-pair stride stays under the 32767-element ISA step bound):
    x   [128, 16, C]: [lo0..lo7 | hi0..hi7]                   (pair stride 8C)
    wfc [128, 16, H]: [h0..h3 l0..l3 | h4..h7 l4..l7]         (stride 4H=16384)
    a   [128, 64, S]: [hi0..hi31 | lo0..lo31]                 (stride 32S)
    wpr [128, 64, D]: [lo0..15 hi0..15 | lo16..31 hi16..31]   (stride 16D=16384)
Cross pairs come from "(grp two k) -> grp k two" rearrange views; speed pairs
are contiguous plane pairs within each hi block.

Weights are pre-scaled by 64 so fp8 stays in normal range; mm1 dequant folds
into the Prelu input scale, mm2 dequant folds into the host-side gate values.
a_hi/a_lo are produced on device: ScalarE Prelu -> Square -> Copy(->fp8), and
a VectorE subtract for the residual.
"""

import numpy as np
import ml_dtypes
from contextlib import ExitStack

B, T, D, H, E = 4, 2048, 1024, 4096, 8
N = B * T
P = 128
CHUNK = 512
SW = 64.0  # weight pre-scale so e4m3 stays in normal range

F8 = ml_dtypes.float8_e4m3

_NC_CACHE = {}


def _build_nc(C):
    """Per-core Bass program for capacity C tokens (C % 128 == 0)."""
    if C in _NC_CACHE:
        return _NC_CACHE[C]
    import concourse.bacc as bacc
    import concourse.tile as tile
    import concourse.mybir as mybir

    assert C % P == 0
    f8 = mybir.dt.float8e4
    f16 = mybir.dt.float16
    f32 = mybir.dt.float32
    AF = mybir.ActivationFunctionType
    DR = mybir.MatmulPerfMode.DoubleRow

    KD = D // P          # 8  k-blocks for mm1
    KH = H // P          # 32 k-blocks for mm2 (h-blocks of mm1 output)
    DN = D // CHUNK      # 2 output-column blocks for mm2

    nc = bacc.Bacc(None, target_bir_lowering=False, debug=False)
    xq = nc.dram_tensor("xq", [P, 2 * KD, C], f8, kind="ExternalInput")
    wfcq = nc.dram_tensor("wfcq", [P, 2 * KD, H], f8, kind="ExternalInput")
    wprojq = nc.dram_tensor("wprojq", [P, 2 * KH, D], f8, kind="ExternalInput")
    g = nc.dram_tensor("g", [P, C // P], f32, kind="ExternalInput")
    out = nc.dram_tensor("outp", [C, D], f32, kind="ExternalOutput")
    out_v = out.ap().rearrange("(c p) d -> p c d", p=P)          # [128, C//128, D]

    chunks = [CHUNK] * (C // CHUNK)
    if C % CHUNK:
        chunks.append(C % CHUNK)

    with tile.TileContext(nc) as tc:
        with ExitStack() as ctx:
            const = ctx.enter_context(tc.tile_pool(name="const", bufs=1))
            xpool = ctx.enter_context(tc.tile_pool(name="xp", bufs=2))
            apool = ctx.enter_context(tc.tile_pool(name="apool", bufs=1))
            ppool = ctx.enter_context(tc.tile_pool(name="pp", bufs=4))
            opool = ctx.enter_context(tc.tile_pool(name="op", bufs=4))
            ps1pool = ctx.enter_context(tc.tile_pool(name="ps1", bufs=3, space="PSUM"))
            ps2pool = ctx.enter_context(tc.tile_pool(name="ps2", bufs=4, space="PSUM"))
            warmpool = ctx.enter_context(tc.tile_pool(name="wm", bufs=1, space="PSUM"))

            # Startup-critical DMAs first: mm1 (mh=0) needs wfc cols 0:128
            # (all 16 planes) and x chunk-0 (hi planes first: the speed
            # passes run before the cross passes).
            x_tiles = {}
            x_tiles[0] = xpool.tile([P, 2 * KD, chunks[0]], f8, tag="xt", name="xt0")
            wfc_sb = const.tile([P, 2 * KD, H], f8)
            nc.sync.dma_start(wfc_sb[:, :, 0:P], wfcq.ap()[:, :, 0:P])
            nc.sync.dma_start(x_tiles[0][:, KD:2 * KD, :], xq.ap()[:, KD:2 * KD, 0:chunks[0]])
            nc.sync.dma_start(wfc_sb[:, :, P:2 * P], wfcq.ap()[:, :, P:2 * P])
            nc.sync.dma_start(x_tiles[0][:, 0:KD, :], xq.ap()[:, 0:KD, 0:chunks[0]])
            # Rest of wfc in H-slices (mm1 h-block mh only depends on the
            # slices covering its 128 columns), interleaved with wproj
            # slices so wproj is ready when chunk-0 mm2 starts (~41us in).
            wproj_sb = const.tile([P, 2 * KH, D], f8)
            wfc_slices = [(s0, H // 16) for s0 in range(2 * P, H, H // 16)]
            wproj_slices = [
                (kc * (2 * KH // 8), 2 * KH // 8) for kc in range(8)
            ]
            wi = 0
            for s0, w in wfc_slices:
                nc.sync.dma_start(wfc_sb[:, :, s0:s0 + w], wfcq.ap()[:, :, s0:s0 + w])
                if wi < len(wproj_slices):
                    p0, pw = wproj_slices[wi]
                    nc.sync.dma_start(
                        wproj_sb[:, p0:p0 + pw, :], wprojq.ap()[:, p0:p0 + pw, :]
                    )
                    wi += 1
            for p0, pw in wproj_slices[wi:]:
                nc.sync.dma_start(wproj_sb[:, p0:p0 + pw, :], wprojq.ap()[:, p0:p0 + pw, :])
            g_sb = const.tile([P, C // P], f32)
            nc.sync.dma_start(g_sb[:], g.ap())

            # PE warmup: burn the DMA-wait window on dummy matmuls so the
            # clock-gate grants full rate when real work starts.
            warm_sb = const.tile([P, P], f16)
            nc.vector.memset(warm_sb[:], 0.0)
            warm_ps = warmpool.tile([P, P], f32)
            for _ in range(38):
                nc.tensor.matmul(warm_ps[:], warm_sb[:], warm_sb[:],
                                 start=True, stop=True)

            # Cross-pair views: [:, grp, k] (or [:, k]) is a [128, 2, .] AP
            # whose pair stride is the group's hi->lo plane distance.
            wfc_cross = wfc_sb[:].rearrange(
                "p (grp two k) h -> p grp k two h", grp=2, two=2
            )
            wproj_cross = wproj_sb[:].rearrange(
                "p (grp two k) d -> p grp k two d", grp=2, two=2
            )
            tok0 = 0
            for c, S in enumerate(chunks):
                if c not in x_tiles:
                    x_tiles[c] = xpool.tile([P, 2 * KD, S], f8, tag="xt", name=f"xt{c}")
                    nc.sync.dma_start(x_tiles[c][:], xq.ap()[:, :, tok0:tok0 + S])
                x_tile = x_tiles[c]
                x_cross = x_tile[:].rearrange("p (two k) s -> p k two s", two=2)
                a_t = apool.tile([P, 2 * KH, S], f8, tag="at")
                a_cross = a_t[:].rearrange("p (two k) s -> p k two s", two=2)
                for mh in range(KH):
                    ps1 = ps1pool.tile([P, S], f32, tag="ps1")
                    cols = slice(mh * P, (mh + 1) * P)
                    for u in range(KD // 2):
                        # hi planes of wfc group u//2 start at (u//2)*8
                        wb = (u // 2) * 8 + (u % 2) * 2
                        nc.tensor.matmul(
                            ps1[:],
                            wfc_sb[:, wb:wb + 2, cols],
                            x_tile[:, KD + 2 * u:KD + 2 * u + 2, :],
                            start=(u == 0), stop=False, perf_mode=DR,
                        )
                    for k in range(KD):
                        nc.tensor.matmul(
                            ps1[:],
                            wfc_cross[:, k // 4, k % 4, :, cols],
                            x_cross[:, k, :, :],
                            start=False, stop=(k == KD - 1), perf_mode=DR,
                        )
                    # p = prelu(h, 0.5) with the 1/SW dequant folded in;
                    # a16 = p^2; a_hi = e4m3(a16); a_lo = a16 - a_hi.
                    # Spread across ScalarE / DVE / Pool so no engine exceeds
                    # the PE's per-h-block pace (ScalarE alone is too slow).
                    p16 = ppool.tile([P, S], f16, tag="p16")
                    nc.scalar.activation(p16[:], ps1[:], AF.Prelu,
                                         alpha=0.5, scale=1.0 / SW)
                    a16 = ppool.tile([P, S], f16, tag="a16")
                    nc.vector.tensor_tensor(
                        a16[:], p16[:], p16[:], mybir.AluOpType.mult,
                    )
                    nc.gpsimd.tensor_copy(a_t[:, mh, :], a16[:])
                    nc.gpsimd.tensor_tensor(
                        a_t[:, KH + mh, :], a16[:], a_t[:, mh, :],
                        mybir.AluOpType.subtract,
                    )
                for ti in range(S // P):
                    gcol = tok0 // P + ti
                    acols = slice(ti * P, (ti + 1) * P)
                    for dn in range(DN):
                        dcols = slice(dn * CHUNK, (dn + 1) * CHUNK)
                        ps2 = ps2pool.tile([P, CHUNK], f32, tag="ps2")
                        # Cross passes first: cross k only needs a planes
                        # (k, 32+k), so mm2 can start as soon as h-block 0
                        # is evicted instead of waiting for the full a tile.
                        for k in range(KH):
                            nc.tensor.matmul(
                                ps2[:],
                                a_cross[:, k, :, acols],
                                wproj_cross[:, k // 16, k % 16, :, dcols],
                                start=(k == 0), stop=False, perf_mode=DR,
                            )
                        for u in range(KH // 2):
                            # hi planes of wproj group u//8 start at
                            # (u//8)*32 + 16
                            wb = (u // 8) * 32 + 16 + (2 * u) % 16
                            nc.tensor.matmul(
                                ps2[:],
                                a_t[:, 2 * u:2 * u + 2, acols],
                                wproj_sb[:, wb:wb + 2, dcols],
                                start=False, stop=(u == KH // 2 - 1),
                                perf_mode=DR,
                            )
                        o_tile = opool.tile([P, CHUNK], f32, tag="ot")
                        # fused gate+dequant: out = psum * (g[token]/SW)
                        nc.scalar.activation(
                            o_tile[:], ps2[:], AF.Copy,
                            scale=g_sb[:, gcol:gcol + 1],
                        )
                        nc.sync.dma_start(
                            out_v[:, gcol, dn * CHUNK:(dn + 1) * CHUNK], o_tile[:]
                        )
                tok0 += S
    nc.compile()
    _NC_CACHE[C] = nc
    return nc


def _route(xf, Wg):
    """Exact top-2 gating in fp32, mirroring the reference math."""
    logits = xf @ Wg.T                                   # [N, E]
    top2 = np.argpartition(logits, E - 2, axis=1)[:, E - 2:]   # [N, 2] unordered
    vals = np.take_along_axis(logits, top2, axis=1)
    m = vals.max(axis=1, keepdims=True)
    ex = np.exp(vals - m)
    w = ex / ex.sum(axis=1, keepdims=True)               # [N, 2] softmax over top-2
    return top2, w


def _split8(v):
    """fp32 -> (hi, lo) e4m3 pair with hi = e4m3(v), lo = e4m3(v - hi)."""
    hi = v.astype(F8)
    lo = (v - hi.astype(np.float32)).astype(F8)
    return hi, lo


_WPACK_CACHE = {}


def _pack_weights(Wfc, Wproj):
    key = (Wfc.ctypes.data, Wproj.ctypes.data, Wfc.shape, Wproj.shape)
    if key in _WPACK_CACHE:
        return _WPACK_CACHE[key]
    KD, KH = D // P, H // P
    packed = []
    for e in range(E):
        wfcT = np.ascontiguousarray(Wfc[e].T.astype(np.float32) * SW)   # [D, H]
        wh, wl = _split8(wfcT)
        whp = wh.reshape(KD, P, H).transpose(1, 0, 2)                   # [128,8,H]
        wlp = wl.reshape(KD, P, H).transpose(1, 0, 2)
        wfcq = np.concatenate(
            [whp[:, 0:4], wlp[:, 0:4], whp[:, 4:8], wlp[:, 4:8]], axis=1)
        wprojT = np.ascontiguousarray(Wproj[e].T.astype(np.float32) * SW)  # [H, D]
        w2h, w2l = _split8(wprojT)
        w2hp = w2h.reshape(KH, P, D).transpose(1, 0, 2)                 # [128,32,D]
        w2lp = w2l.reshape(KH, P, D).transpose(1, 0, 2)
        wprojq = np.concatenate(
            [w2lp[:, 0:16], w2hp[:, 0:16], w2lp[:, 16:32], w2hp[:, 16:32]],
            axis=1)                                                     # [128,64,D]
        packed.append((np.ascontiguousarray(wfcq), np.ascontiguousarray(wprojq)))
    _WPACK_CACHE[key] = packed
    return packed


def run_moe(x, Wg, Wfc, Wproj, trace=False):
    from concourse import bass_utils

    xf = np.ascontiguousarray(x.reshape(-1, D), dtype=np.float32)
    top2, w = _route(xf, Wg.astype(np.float32))

    toks, gates = [], []
    for e in range(E):
        sel = np.nonzero((top2 == e).any(axis=1))[0]
        ge = (w[sel] * (top2[sel] == e)).sum(axis=1).astype(np.float32)
        toks.append(sel)
        gates.append(ge)

    maxc = max(len(t) for t in toks)
    C = max(P, ((maxc + P - 1) // P) * P)

    nc = _build_nc(C)
    wpacked = _pack_weights(Wfc, Wproj)

    KD = D // P
    in_maps = []
    for e in range(E):
        te = toks[e]
        xT_e = np.zeros((D, C), np.float32)
        xT_e[:, :len(te)] = xf[te].T
        xh, xl = _split8(xT_e)
        xq_e = np.concatenate(
            [xl.reshape(KD, P, C).transpose(1, 0, 2),
             xh.reshape(KD, P, C).transpose(1, 0, 2)], axis=1)          # [128,16,C]
        g_e = np.zeros((C,), np.float32)
        g_e[:len(te)] = gates[e] * (1.0 / SW)
        g_mat = np.ascontiguousarray(g_e.reshape(C // P, P).T)
        in_maps.append({
            "xq": np.ascontiguousarray(xq_e),
            "wfcq": wpacked[e][0],
            "wprojq": wpacked[e][1],
            "g": g_mat,
        })

    res = bass_utils.run_bass_kernel_spmd(
        nc, in_maps, core_ids=list(range(E)), trace=False
    )

    out = np.zeros((N, D), np.float32)
    for e in range(E):
        te = toks[e]
        out[te] += res.results[e]["outp"][:len(te)]
    return out.reshape(B, T, D), res


def kernel(x, Wg, Wfc, Wproj):
    out, _ = run_moe(np.asarray(x), np.asarray(Wg), np.asarray(Wfc), np.asarray(Wproj))
    return out


# revision 13
# speedup vs baseline: 1.2600x; 1.2600x over previous
"""MoE MLP (top-2 of 8 experts) Trainium2 kernel.

Strategy: expert-parallel across the 8 NeuronCores (host does the exact fp32
top-2 gating and per-expert token gather, as before), but both big matmuls run
as fp8e4m3 DoubleRow with a hi/lo split:

    v = v_hi + v_lo,  v_hi = e4m3(v),  v_lo = e4m3(v - v_hi)

Per 256 contraction rows, three DoubleRow passes accumulate into PSUM:
    speed pass : pairs (w_hi[2u], w_hi[2u+1]) x (x_hi[2u], x_hi[2u+1])
    cross pass : pairs (w_hi[k],  w_lo[k])   x (x_lo[k],  x_hi[k])   (x2)
which computes sum(x_hi*w_hi + x_lo*w_hi + x_hi*w_lo) - exact up to the
dropped lo*lo term (~1e-3 relative). DoubleRow fp8 runs the PE at 2x fp16
rate per pass, so 3 passes per 256 rows = 0.75x the fp16 matmul time, with
near-fp16 accuracy (measured rel err ~2e-3 end to end).

SBUF plane layout (no strided pair slices needed; hi/lo halves are grouped
so every cross-pair stride stays under the 32767-element ISA step bound):
    x   [128, 16, C]: [lo0..lo7 | hi0..hi7]                   (pair stride 8C)
    wfc [128, 16, H]: [h0..h3 l0..l3 | h4..h7 l4..l7]         (stride 4H=16384)
    a   [128, 64, S]: [hi0..hi31 | lo0..lo31]                 (stride 32S)
    wpr [128, 64, D]: [lo0..15 hi0..15 | lo16..31 hi16..31]   (stride 16D=16384)
Cross pairs come from "(grp two k) -> grp k two" rearrange views; speed pairs
are contiguous plane pairs within each hi block.

Weights are pre-scaled by 64 so fp8 stays in normal range; mm1 dequant folds
into the Prelu input scale, mm2 dequant folds into the host-side gate values.
a_hi/a_lo are produced on device: ScalarE Prelu -> Square -> Copy(->fp8), and
a VectorE subtract for the residual.
"""

import numpy as np
import ml_dtypes
from contextlib import ExitStack

B, T, D, H, E = 4, 2048, 1024, 4096, 8
N = B * T
P = 128
CHUNK = 512
SW = 64.0  # weight pre-scale so e4m3 stays in normal range

F8 = ml_dtypes.float8_e4m3

_NC_CACHE = {}


def _build_nc(C):
    """Per-core Bass program for capacity C tokens (C % 128 == 0)."""
    if C in _NC_CACHE:
        return _NC_CACHE[C]
    import concourse.bacc as bacc
    import concourse.tile as tile
    import concourse.mybir as mybir

    assert C % P == 0
    f8 = mybir.dt.float8e4
    f16 = mybir.dt.float16
    f32 = mybir.dt.float32
    AF = mybir.ActivationFunctionType
    DR = mybir.MatmulPerfMode.DoubleRow

    KD = D // P          # 8  k-blocks for mm1
    KH = H // P          # 32 k-blocks for mm2 (h-blocks of mm1 output)
    DN = D // CHUNK      # 2 output-column blocks for mm2

    nc = bacc.Bacc(None, target_bir_lowering=False, debug=False)
    xq = nc.dram_tensor("xq", [P, 2 * KD, C], f8, kind="ExternalInput")
    wfcq = nc.dram_tensor("wfcq", [P, 2 * KD, H], f8, kind="ExternalInput")
    wprojq = nc.dram_tensor("wprojq", [P, 2 * KH, D], f8, kind="ExternalInput")
    g = nc.dram_tensor("g", [P, C // P], f32, kind="ExternalInput")
    out = nc.dram_tensor("outp", [C, D], f32, kind="ExternalOutput")
    out_v = out.ap().rearrange("(c p) d -> p c d", p=P)          # [128, C//128, D]

    chunks = [CHUNK] * (C // CHUNK)
    if C % CHUNK:
        chunks.append(C % CHUNK)

    with tile.TileContext(nc) as tc:
        with ExitStack() as ctx:
            const = ctx.enter_context(tc.tile_pool(name="const", bufs=1))
            xpool = ctx.enter_context(tc.tile_pool(name="xp", bufs=2))
            apool = ctx.enter_context(tc.tile_pool(name="apool", bufs=1))
            ppool = ctx.enter_context(tc.tile_pool(name="pp", bufs=4))
            opool = ctx.enter_context(tc.tile_pool(name="op", bufs=4))
            ps1pool = ctx.enter_context(tc.tile_pool(name="ps1", bufs=3, space="PSUM"))
            ps2pool = ctx.enter_context(tc.tile_pool(name="ps2", bufs=4, space="PSUM"))
            warmpool = ctx.enter_context(tc.tile_pool(name="wm", bufs=1, space="PSUM"))

            # Startup-critical DMAs first: mm1 (mh=0) needs wfc cols 0:128
            # (all 16 planes) and x chunk-0 (hi planes first: the speed
            # passes run before the cross passes). DMA transfers serialize
            # at ~300GB/s in the model, so order = need-by time: all of wfc
            # (consumed column-paced through chunk-0 mm1), then wproj in
            # dn-column halves (mm2 runs dn-outer so the dn=1 half can
            # arrive ~15us later).
            x_tiles = {}
            x_tiles[0] = xpool.tile([P, 2 * KD, chunks[0]], f8, tag="xt", name="xt0")
            wfc_sb = const.tile([P, 2 * KD, H], f8)
            nc.sync.dma_start(wfc_sb[:, :, 0:P], wfcq.ap()[:, :, 0:P])
            nc.sync.dma_start(x_tiles[0][:, KD:2 * KD, :], xq.ap()[:, KD:2 * KD, 0:chunks[0]])
            nc.sync.dma_start(wfc_sb[:, :, P:2 * P], wfcq.ap()[:, :, P:2 * P])
            nc.sync.dma_start(x_tiles[0][:, 0:KD, :], xq.ap()[:, 0:KD, 0:chunks[0]])
            for s0 in range(2 * P, H, H // 16):
                w = H // 16
                nc.sync.dma_start(wfc_sb[:, :, s0:s0 + w], wfcq.ap()[:, :, s0:s0 + w])
            wproj_sb = const.tile([P, 2 * KH, D], f8)
            for dn in range(DN):
                dcols = slice(dn * CHUNK, (dn + 1) * CHUNK)
                for kc in range(4):
                    sl = slice(kc * (2 * KH // 4), (kc + 1) * (2 * KH // 4))
                    nc.sync.dma_start(wproj_sb[:, sl, dcols], wprojq.ap()[:, sl, dcols])
            g_sb = const.tile([P, C // P], f32)
            nc.sync.dma_start(g_sb[:], g.ap())

            # PE warmup: burn the DMA-wait window on dummy matmuls so the
            # clock-gate grants full rate when real work starts.
            warm_sb = const.tile([P, P], f16)
            nc.vector.memset(warm_sb[:], 0.0)
            warm_ps = warmpool.tile([P, P], f32)
            for _ in range(38):
                nc.tensor.matmul(warm_ps[:], warm_sb[:], warm_sb[:],
                                 start=True, stop=True)

            # Cross-pair views: [:, grp, k] (or [:, k]) is a [128, 2, .] AP
            # whose pair stride is the group's hi->lo plane distance.
            wfc_cross = wfc_sb[:].rearrange(
                "p (grp two k) h -> p grp k two h", grp=2, two=2
            )
            wproj_cross = wproj_sb[:].rearrange(
                "p (grp two k) d -> p grp k two d", grp=2, two=2
            )
            tok0 = 0
            for c, S in enumerate(chunks):
                if c not in x_tiles:
                    x_tiles[c] = xpool.tile([P, 2 * KD, S], f8, tag="xt", name=f"xt{c}")
                    nc.sync.dma_start(x_tiles[c][:], xq.ap()[:, :, tok0:tok0 + S])
                x_tile = x_tiles[c]
                x_cross = x_tile[:].rearrange("p (two k) s -> p k two s", two=2)
                a_t = apool.tile([P, 2 * KH, S], f8, tag="at")
                a_cross = a_t[:].rearrange("p (two k) s -> p k two s", two=2)
                for mh in range(KH):
                    ps1 = ps1pool.tile([P, S], f32, tag="ps1")
                    cols = slice(mh * P, (mh + 1) * P)
                    for u in range(KD // 2):
                        # hi planes of wfc group u//2 start at (u//2)*8
                        wb = (u // 2) * 8 + (u % 2) * 2
                        nc.tensor.matmul(
                            ps1[:],
                            wfc_sb[:, wb:wb + 2, cols],
                            x_tile[:, KD + 2 * u:KD + 2 * u + 2, :],
                            start=(u == 0), stop=False, perf_mode=DR,
                        )
                    for k in range(KD):
                        nc.tensor.matmul(
                            ps1[:],
                            wfc_cross[:, k // 4, k % 4, :, cols],
                            x_cross[:, k, :, :],
                            start=False, stop=(k == KD - 1), perf_mode=DR,
                        )
                    # p = prelu(h, 0.5) with the 1/SW dequant folded in;
                    # a16 = p^2; a_hi = e4m3(a16); a_lo = a16 - a_hi.
                    # Spread across ScalarE / DVE / Pool so no engine exceeds
                    # the PE's per-h-block pace (ScalarE alone is too slow).
                    p16 = ppool.tile([P, S], f16, tag="p16")
                    nc.scalar.activation(p16[:], ps1[:], AF.Prelu,
                                         alpha=0.5, scale=1.0 / SW)
                    a16 = ppool.tile([P, S], f16, tag="a16")
                    nc.vector.tensor_tensor(
                        a16[:], p16[:], p16[:], mybir.AluOpType.mult,
                    )
                    nc.scalar.activation(a_t[:, mh, :], a16[:], AF.Copy)
                    nc.vector.tensor_tensor(
                        a_t[:, KH + mh, :], a16[:], a_t[:, mh, :],
                        mybir.AluOpType.subtract,
                    )
                for dn in range(DN):
                    dcols = slice(dn * CHUNK, (dn + 1) * CHUNK)
                    for ti in range(S // P):
                        gcol = tok0 // P + ti
                        acols = slice(ti * P, (ti + 1) * P)
                        ps2 = ps2pool.tile([P, CHUNK], f32, tag="ps2")
                        # Cross passes first: cross k only needs a planes
                        # (k, 32+k), so mm2 can start as soon as h-block 0
                        # is evicted instead of waiting for the full a tile.
                        for k in range(KH):
                            nc.tensor.matmul(
                                ps2[:],
                                a_cross[:, k, :, acols],
                                wproj_cross[:, k // 16, k % 16, :, dcols],
                                start=(k == 0), stop=False, perf_mode=DR,
                            )
                        for u in range(KH // 2):
                            # hi planes of wproj group u//8 start at
                            # (u//8)*32 + 16
                            wb = (u // 8) * 32 + 16 + (2 * u) % 16
                            nc.tensor.matmul(
                                ps2[:],
                                a_t[:, 2 * u:2 * u + 2, acols],
                                wproj_sb[:, wb:wb + 2, dcols],
                                start=False, stop=(u == KH // 2 - 1),
                                perf_mode=DR,
                            )
                        o_tile = opool.tile([P, CHUNK], f32, tag="ot")
                        # fused gate+dequant: out = psum * (g[token]/SW)
                        nc.scalar.activation(
                            o_tile[:], ps2[:], AF.Copy,
                            scale=g_sb[:, gcol:gcol + 1],
                        )
                        nc.sync.dma_start(
                            out_v[:, gcol, dn * CHUNK:(dn + 1) * CHUNK], o_tile[:]
                        )
                tok0 += S
    nc.compile()
    _NC_CACHE[C] = nc
    return nc


def _route(xf, Wg):
    """Exact top-2 gating in fp32, mirroring the reference math."""
    logits = xf @ Wg.T                                   # [N, E]
    top2 = np.argpartition(logits, E - 2, axis=1)[:, E - 2:]   # [N, 2] unordered
    vals = np.take_along_axis(logits, top2, axis=1)
    m = vals.max(axis=1, keepdims=True)
    ex = np.exp(vals - m)
    w = ex / ex.sum(axis=1, keepdims=True)               # [N, 2] softmax over top-2
    return top2, w


def _split8(v):
    """fp32 -> (hi, lo) e4m3 pair with hi = e4m3(v), lo = e4m3(v - hi)."""
    hi = v.astype(F8)
    lo = (v - hi.astype(np.float32)).astype(F8)
    return hi, lo


_WPACK_CACHE = {}


def _pack_weights(Wfc, Wproj):
    key = (Wfc.ctypes.data, Wproj.ctypes.data, Wfc.shape, Wproj.shape)
    if key in _WPACK_CACHE:
        return _WPACK_CACHE[key]
    KD, KH = D // P, H // P
    packed = []
    for e in range(E):
        wfcT = np.ascontiguousarray(Wfc[e].T.astype(np.float32) * SW)   # [D, H]
        wh, wl = _split8(wfcT)
        whp = wh.reshape(KD, P, H).transpose(1, 0, 2)                   # [128,8,H]
        wlp = wl.reshape(KD, P, H).transpose(1, 0, 2)
        wfcq = np.concatenate(
            [whp[:, 0:4], wlp[:, 0:4], whp[:, 4:8], wlp[:, 4:8]], axis=1)
        wprojT = np.ascontiguousarray(Wproj[e].T.astype(np.float32) * SW)  # [H, D]
        w2h, w2l = _split8(wprojT)
        w2hp = w2h.reshape(KH, P, D).transpose(1, 0, 2)                 # [128,32,D]
        w2lp = w2l.reshape(KH, P, D).transpose(1, 0, 2)
        wprojq = np.concatenate(
            [w2lp[:, 0:16], w2hp[:, 0:16], w2lp[:, 16:32], w2hp[:, 16:32]],
            axis=1)                                                     # [128,64,D]
        packed.append((np.ascontiguousarray(wfcq), np.ascontiguousarray(wprojq)))
    _WPACK_CACHE[key] = packed
    return packed


def run_moe(x, Wg, Wfc, Wproj, trace=False):
    from concourse import bass_utils

    xf = np.ascontiguousarray(x.reshape(-1, D), dtype=np.float32)
    top2, w = _route(xf, Wg.astype(np.float32))

    toks, gates = [], []
    for e in range(E):
        sel = np.nonzero((top2 == e).any(axis=1))[0]
        ge = (w[sel] * (top2[sel] == e)).sum(axis=1).astype(np.float32)
        toks.append(sel)
        gates.append(ge)

    maxc = max(len(t) for t in toks)
    C = max(P, ((maxc + P - 1) // P) * P)

    nc = _build_nc(C)
    wpacked = _pack_weights(Wfc, Wproj)

    KD = D // P
    in_maps = []
    for e in range(E):
        te = toks[e]
        xT_e = np.zeros((D, C), np.float32)
        xT_e[:, :len(te)] = xf[te].T
        xh, xl = _split8(xT_e)
        xq_e = np.concatenate(
            [xl.reshape(KD, P, C).transpose(1, 0, 2),
             xh.reshape(KD, P, C).transpose(1, 0, 2)], axis=1)          # [128,16,C]
        g_e = np.zeros((C,), np.float32)
        g_e[:len(te)] = gates[e] * (1.0 / SW)
        g_mat = np.ascontiguousarray(g_e.reshape(C // P, P).T)
        in_maps.append({
            "xq": np.ascontiguousarray(xq_e),
            "wfcq": wpacked[e][0],
            "wprojq": wpacked[e][1],
            "g": g_mat,
        })

    res = bass_utils.run_bass_kernel_spmd(
        nc, in_maps, core_ids=list(range(E)), trace=False
    )

    out = np.zeros((N, D), np.float32)
    for e in range(E):
        te = toks[e]
        out[te] += res.results[e]["outp"][:len(te)]
    return out.reshape(B, T, D), res


def kernel(x, Wg, Wfc, Wproj):
    out, _ = run_moe(np.asarray(x), np.asarray(Wg), np.asarray(Wfc), np.asarray(Wproj))
    return out


# revision 18
# speedup vs baseline: 1.2709x; 1.0087x over previous
"""MoE MLP (top-2 of 8 experts) Trainium2 kernel.

Strategy: expert-parallel across the 8 NeuronCores (host does the exact fp32
top-2 gating and per-expert token gather, as before), but both big matmuls run
as fp8e4m3 DoubleRow with a hi/lo split:

    v = v_hi + v_lo,  v_hi = e4m3(v),  v_lo = e4m3(v - v_hi)

Per 256 contraction rows, three DoubleRow passes accumulate into PSUM:
    speed pass : pairs (w_hi[2u], w_hi[2u+1]) x (x_hi[2u], x_hi[2u+1])
    cross pass : pairs (w_hi[k],  w_lo[k])   x (x_lo[k],  x_hi[k])   (x2)
which computes sum(x_hi*w_hi + x_lo*w_hi + x_hi*w_lo) - exact up to the
dropped lo*lo term (~1e-3 relative). DoubleRow fp8 runs the PE at 2x fp16
rate per pass, so 3 passes per 256 rows = 0.75x the fp16 matmul time, with
near-fp16 accuracy (measured rel err ~2e-3 end to end).

SBUF plane layout (no strided pair slices needed; hi/lo halves are grouped
so every cross-pair stride stays under the 32767-element ISA step bound):
    x   [128, 16, C]: [lo0..lo7 | hi0..hi7]                   (pair stride 8C)
    wfc [128, 16, H]: [h0..h3 l0..l3 | h4..h7 l4..l7]         (stride 4H=16384)
    a   [128, 64, S]: [hi0..hi31 | lo0..lo31]                 (stride 32S)
    wpr [128, 64, D]: [lo0..15 hi0..15 | lo16..31 hi16..31]   (stride 16D=16384)
Cross pairs come from "(grp two k) -> grp k two" rearrange views; speed pairs
are contiguous plane pairs within each hi block.

Weights are pre-scaled by 64 so fp8 stays in normal range; mm1 dequant folds
into the Prelu input scale, mm2 dequant folds into the host-side gate values.
a_hi/a_lo are produced on device: ScalarE Prelu -> Square -> Copy(->fp8), and
a VectorE subtract for the residual.
"""

import numpy as np
import ml_dtypes
from contextlib import ExitStack

B, T, D, H, E = 4, 2048, 1024, 4096, 8
N = B * T
P = 128
CHUNK = 512
SW = 64.0  # weight pre-scale so e4m3 stays in normal range

F8 = ml_dtypes.float8_e4m3

_NC_CACHE = {}


def _build_nc(C, CT):
    """Per-core Bass program: buffers sized for C tokens (C % 128 == 0), but
    compute trimmed to CT tokens (CT % 32 == 0, CT <= C) — the 128-granular
    buffer padding costs only DMA bytes, not PE time."""
    if (C, CT) in _NC_CACHE:
        return _NC_CACHE[(C, CT)]
    import concourse.bacc as bacc
    import concourse.tile as tile
    import concourse.mybir as mybir

    assert C % P == 0 and CT % 32 == 0 and CT <= C
    f8 = mybir.dt.float8e4
    f16 = mybir.dt.float16
    f32 = mybir.dt.float32
    AF = mybir.ActivationFunctionType
    DR = mybir.MatmulPerfMode.DoubleRow

    KD = D // P          # 8  k-blocks for mm1
    KH = H // P          # 32 k-blocks for mm2 (h-blocks of mm1 output)
    DN = D // CHUNK      # 2 output-column blocks for mm2

    nc = bacc.Bacc(None, target_bir_lowering=False, debug=False)
    xq = nc.dram_tensor("xq", [P, 2 * KD, C], f8, kind="ExternalInput")
    wfcq = nc.dram_tensor("wfcq", [P, 2 * KD, H], f8, kind="ExternalInput")
    wprojq = nc.dram_tensor("wprojq", [P, 2 * KH, D], f8, kind="ExternalInput")
    g = nc.dram_tensor("g", [P, C // P], f32, kind="ExternalInput")
    out = nc.dram_tensor("outp", [C, D], f32, kind="ExternalOutput")
    out_v = out.ap().rearrange("(c p) d -> p c d", p=P)          # [128, C//128, D]

    chunks = [CHUNK] * (CT // CHUNK)
    if CT % CHUNK:
        chunks.append(CT % CHUNK)

    with tile.TileContext(nc) as tc:
        with ExitStack() as ctx:
            const = ctx.enter_context(tc.tile_pool(name="const", bufs=1))
            xpool = ctx.enter_context(tc.tile_pool(name="xp", bufs=2))
            apool = ctx.enter_context(tc.tile_pool(name="apool", bufs=1))
            ppool = ctx.enter_context(tc.tile_pool(name="pp", bufs=4))
            opool = ctx.enter_context(tc.tile_pool(name="op", bufs=4))
            ps1pool = ctx.enter_context(tc.tile_pool(name="ps1", bufs=3, space="PSUM"))
            ps2pool = ctx.enter_context(tc.tile_pool(name="ps2", bufs=4, space="PSUM"))
            warmpool = ctx.enter_context(tc.tile_pool(name="wm", bufs=1, space="PSUM"))

            # Startup-critical DMAs first: mm1 (mh=0) needs wfc cols 0:128
            # (all 16 planes) and x chunk-0 (hi planes first: the speed
            # passes run before the cross passes). DMA transfers serialize
            # at ~300GB/s in the model, so order = need-by time: all of wfc
            # (consumed column-paced through chunk-0 mm1), then wproj in
            # dn-column halves (mm2 runs dn-outer so the dn=1 half can
            # arrive ~15us later).
            x_tiles = {}
            x_tiles[0] = xpool.tile([P, 2 * KD, chunks[0]], f8, tag="xt", name="xt0")
            wfc_sb = const.tile([P, 2 * KD, H], f8)
            nc.sync.dma_start(wfc_sb[:, :, 0:P], wfcq.ap()[:, :, 0:P])
            nc.sync.dma_start(x_tiles[0][:, KD:2 * KD, :], xq.ap()[:, KD:2 * KD, 0:chunks[0]])
            nc.sync.dma_start(x_tiles[0][:, 0:KD, :], xq.ap()[:, 0:KD, 0:chunks[0]])
            nc.sync.dma_start(wfc_sb[:, :, P:2 * P], wfcq.ap()[:, :, P:2 * P])
            for s0 in range(2 * P, H, H // 16):
                w = H // 16
                nc.sync.dma_start(wfc_sb[:, :, s0:s0 + w], wfcq.ap()[:, :, s0:s0 + w])
            # wproj per dn-half in 8-plane slices, ordered so the cross
            # passes' (lo_k, hi_k) pairs become usable earliest.
            wproj_sb = const.tile([P, 2 * KH, D], f8)
            plane_order = [0, 16, 8, 24, 32, 48, 40, 56]
            for dn in range(DN):
                dcols = slice(dn * CHUNK, (dn + 1) * CHUNK)
                for p0 in plane_order:
                    sl = slice(p0, p0 + 8)
                    nc.sync.dma_start(wproj_sb[:, sl, dcols], wprojq.ap()[:, sl, dcols])
            g_sb = const.tile([P, C // P], f32)
            nc.sync.dma_start(g_sb[:], g.ap())

            # PE warmup: burn the DMA-wait window on dummy matmuls so the
            # clock-gate grants full rate when real work starts.
            warm_sb = const.tile([P, P], f16)
            nc.vector.memset(warm_sb[:], 0.0)
            warm_ps = warmpool.tile([P, P], f32)
            for _ in range(38):
                nc.tensor.matmul(warm_ps[:], warm_sb[:], warm_sb[:],
                                 start=True, stop=True)

            # Cross-pair views: [:, grp, k] (or [:, k]) is a [128, 2, .] AP
            # whose pair stride is the group's hi->lo plane distance.
            wfc_cross = wfc_sb[:].rearrange(
                "p (grp two k) h -> p grp k two h", grp=2, two=2
            )
            wproj_cross = wproj_sb[:].rearrange(
                "p (grp two k) d -> p grp k two d", grp=2, two=2
            )
            tok0 = 0
            for c, S in enumerate(chunks):
                if c not in x_tiles:
                    x_tiles[c] = xpool.tile([P, 2 * KD, S], f8, tag="xt", name=f"xt{c}")
                    nc.sync.dma_start(x_tiles[c][:], xq.ap()[:, :, tok0:tok0 + S])
                x_tile = x_tiles[c]
                x_cross = x_tile[:].rearrange("p (two k) s -> p k two s", two=2)
                a_t = apool.tile([P, 2 * KH, S], f8, tag="at")
                a_cross = a_t[:].rearrange("p (two k) s -> p k two s", two=2)
                for mh in range(KH):
                    ps1 = ps1pool.tile([P, S], f32, tag="ps1")
                    cols = slice(mh * P, (mh + 1) * P)
                    for u in range(KD // 2):
                        # hi planes of wfc group u//2 start at (u//2)*8
                        wb = (u // 2) * 8 + (u % 2) * 2
                        nc.tensor.matmul(
                            ps1[:],
                            wfc_sb[:, wb:wb + 2, cols],
                            x_tile[:, KD + 2 * u:KD + 2 * u + 2, :],
                            start=(u == 0), stop=False, perf_mode=DR,
                        )
                    for k in range(KD):
                        nc.tensor.matmul(
                            ps1[:],
                            wfc_cross[:, k // 4, k % 4, :, cols],
                            x_cross[:, k, :, :],
                            start=False, stop=(k == KD - 1), perf_mode=DR,
                        )
                    # p = prelu(h, 0.5) with the 1/SW dequant folded in;
                    # a16 = p^2; a_hi = e4m3(a16); a_lo = a16 - a_hi.
                    # Spread across ScalarE / DVE / Pool so no engine exceeds
                    # the PE's per-h-block pace (ScalarE alone is too slow).
                    p16 = ppool.tile([P, S], f16, tag="p16")
                    nc.scalar.activation(p16[:], ps1[:], AF.Prelu,
                                         alpha=0.5, scale=1.0 / SW)
                    a16 = ppool.tile([P, S], f16, tag="a16")
                    nc.vector.tensor_tensor(
                        a16[:], p16[:], p16[:], mybir.AluOpType.mult,
                    )
                    nc.scalar.activation(a_t[:, mh, :], a16[:], AF.Copy)
                    nc.vector.tensor_tensor(
                        a_t[:, KH + mh, :], a16[:], a_t[:, mh, :],
                        mybir.AluOpType.subtract,
                    )
                for dn in range(DN):
                    dcols = slice(dn * CHUNK, (dn + 1) * CHUNK)
                    for t0 in range(0, S, P):
                        TP = min(P, S - t0)        # ragged tail block (e.g. 32)
                        gcol = (tok0 + t0) // P
                        acols = slice(t0, t0 + TP)
                        ps2 = ps2pool.tile([TP, CHUNK], f32, tag="ps2")
                        # Cross passes first: cross k only needs a planes
                        # (k, 32+k), so mm2 can start as soon as h-block 0
                        # is evicted instead of waiting for the full a tile.
                        for k in range(KH):
                            nc.tensor.matmul(
                                ps2[:],
                                a_cross[:, k, :, acols],
                                wproj_cross[:, k // 16, k % 16, :, dcols],
                                start=(k == 0), stop=False, perf_mode=DR,
                            )
                        for u in range(KH // 2):
                            # hi planes of wproj group u//8 start at
                            # (u//8)*32 + 16
                            wb = (u // 8) * 32 + 16 + (2 * u) % 16
                            nc.tensor.matmul(
                                ps2[:],
                                a_t[:, 2 * u:2 * u + 2, acols],
                                wproj_sb[:, wb:wb + 2, dcols],
                                start=False, stop=(u == KH // 2 - 1),
                                perf_mode=DR,
                            )
                        o_tile = opool.tile([TP, CHUNK], f32, tag="ot")
                        # fused gate+dequant: out = psum * (g[token]/SW)
                        nc.scalar.activation(
                            o_tile[:], ps2[:], AF.Copy,
                            scale=g_sb[0:TP, gcol:gcol + 1],
                        )
                        nc.sync.dma_start(
                            out_v[0:TP, gcol, dn * CHUNK:(dn + 1) * CHUNK],
                            o_tile[:],
                        )
                tok0 += S
    nc.compile()
    _NC_CACHE[C] = nc
    return nc


def _route(xf, Wg):
    """Exact top-2 gating in fp32, mirroring the reference math."""
    logits = xf @ Wg.T                                   # [N, E]
    top2 = np.argpartition(logits, E - 2, axis=1)[:, E - 2:]   # [N, 2] unordered
    vals = np.take_along_axis(logits, top2, axis=1)
    m = vals.max(axis=1, keepdims=True)
    ex = np.exp(vals - m)
    w = ex / ex.sum(axis=1, keepdims=True)               # [N, 2] softmax over top-2
    return top2, w


def _split8(v):
    """fp32 -> (hi, lo) e4m3 pair with hi = e4m3(v), lo = e4m3(v - hi)."""
    hi = v.astype(F8)
    lo = (v - hi.astype(np.float32)).astype(F8)
    return hi, lo


_WPACK_CACHE = {}


def _pack_weights(Wfc, Wproj):
    key = (Wfc.ctypes.data, Wproj.ctypes.data, Wfc.shape, Wproj.shape)
    if key in _WPACK_CACHE:
        return _WPACK_CACHE[key]
    KD, KH = D // P, H // P
    packed = []
    for e in range(E):
        wfcT = np.ascontiguousarray(Wfc[e].T.astype(np.float32) * SW)   # [D, H]
        wh, wl = _split8(wfcT)
        whp = wh.reshape(KD, P, H).transpose(1, 0, 2)                   # [128,8,H]
        wlp = wl.reshape(KD, P, H).transpose(1, 0, 2)
        wfcq = np.concatenate(
            [whp[:, 0:4], wlp[:, 0:4], whp[:, 4:8], wlp[:, 4:8]], axis=1)
        wprojT = np.ascontiguousarray(Wproj[e].T.astype(np.float32) * SW)  # [H, D]
        w2h, w2l = _split8(wprojT)
        w2hp = w2h.reshape(KH, P, D).transpose(1, 0, 2)                 # [128,32,D]
        w2lp = w2l.reshape(KH, P, D).transpose(1, 0, 2)
        wprojq = np.concatenate(
            [w2lp[:, 0:16], w2hp[:, 0:16], w2lp[:, 16:32], w2hp[:, 16:32]],
            axis=1)                                                     # [128,64,D]
        packed.append((np.ascontiguousarray(wfcq), np.ascontiguousarray(wprojq)))
    _WPACK_CACHE[key] = packed
    return packed


def run_moe(x, Wg, Wfc, Wproj, trace=False):
    from concourse import bass_utils

    xf = np.ascontiguousarray(x.reshape(-1, D), dtype=np.float32)
    top2, w = _route(xf, Wg.astype(np.float32))

    toks, gates = [], []
    for e in range(E):
        sel = np.nonzero((top2 == e).any(axis=1))[0]
        ge = (w[sel] * (top2[sel] == e)).sum(axis=1).astype(np.float32)
        toks.append(sel)
        gates.append(ge)

    maxc = max(len(t) for t in toks)
    C = max(P, ((maxc + P - 1) // P) * P)
    CT = max(32, ((maxc + 31) // 32) * 32)

    nc = _build_nc(C, CT)
    wpacked = _pack_weights(Wfc, Wproj)

    KD = D // P
    in_maps = []
    for e in range(E):
        te = toks[e]
        xT_e = np.zeros((D, C), np.float32)
        xT_e[:, :len(te)] = xf[te].T
        xh, xl = _split8(xT_e)
        xq_e = np.concatenate(
            [xl.reshape(KD, P, C).transpose(1, 0, 2),
             xh.reshape(KD, P, C).transpose(1, 0, 2)], axis=1)          # [128,16,C]
        g_e = np.zeros((C,), np.float32)
        g_e[:len(te)] = gates[e] * (1.0 / SW)
        g_mat = np.ascontiguousarray(g_e.reshape(C // P, P).T)
        in_maps.append({
            "xq": np.ascontiguousarray(xq_e),
            "wfcq": wpacked[e][0],
            "wprojq": wpacked[e][1],
            "g": g_mat,
        })

    res = bass_utils.run_bass_kernel_spmd(
        nc, in_maps, core_ids=list(range(E)), trace=False
    )

    out = np.zeros((N, D), np.float32)
    for e in range(E):
        te = toks[e]
        out[te] += res.results[e]["outp"][:len(te)]
    return out.reshape(B, T, D), res


def kernel(x, Wg, Wfc, Wproj):
    out, _ = run_moe(np.asarray(x), np.asarray(Wg), np.asarray(Wfc), np.asarray(Wproj))
    return out
